# revision 5
# baseline (speedup 1.0000x reference)
"""PoolBlockNet Trainium2 kernel.

reference computation (per row n, segment s(n)):
  pos    = permute(end_block_pos_rel)          # (N, 4)
  x      = [h_state | pos] @ W1p.T             # (N, 512)   W1p = [W1[:,32:] | W1[:,:32]@W_sp]
  h1     = relu(segBN(x; g1, be1))             # per-segment batchnorm (biased var)
  y2     = h1 @ W2.T                           # (N, 1024)
  out    = relu(segBN(y2; g2, be2))
Linear biases (b_sp, b1, b2) cancel inside segment BN and are dropped.

Strategy: data-parallel over whole segments across 8 cores.  On-chip layout is
feature-major ([feat, rows]) so BN stats are free-dim reductions; matmuls use
float32r (~1e-4 rounding, 1 cycle/row); the final [1024, rows] -> [rows, 1024]
transpose is done on the PE (exact fp32) and written out contiguously.
"""

import math
import os
import sys

import numpy as np

sys.path.insert(0, "/opt/trn_rl_repo")

import concourse.bacc as bacc
import concourse.bass as bass
import concourse.mybir as mybir
import concourse.tile as tile
from concourse import bass_utils
from concourse.masks import make_identity

F32 = mybir.dt.float32
F32R = mybir.dt.float32r
EPS = 1e-5
H_DIM = 64
D_IN = 68          # 64 h_state + 4 pos features, contraction dim of layer 1
D_MID = 512
D_OUT = 1024
N_CORES = 8

# filled by kernel() for the benchmark harness (exec_time_ns etc.)
LAST_RUN = {}

_PROGRAM_CACHE = {}


def _build_program(rows, seg_len):
    """One-core SPMD program: rows rows, uniform segments of seg_len."""
    if seg_len <= 256:
        assert 512 % seg_len == 0
        chunk = 512
    else:
        assert seg_len <= 512
        chunk = seg_len
    S = chunk // seg_len               # segments per chunk
    assert rows % chunk == 0
    n_chunks = rows // chunk
    G1 = 4 * S                         # (m-tile, seg) groups for layer 1
    G2 = 8 * S

    nc = bacc.Bacc("TRN2", target_bir_lowering=False, debug=False)

    hT = nc.dram_tensor("hT", [H_DIM, rows], F32R, kind="ExternalInput")
    posT = nc.dram_tensor("posT", [4, rows], F32R, kind="ExternalInput")
    w1 = nc.dram_tensor("w1", [D_IN, D_MID], F32R, kind="ExternalInput")
    w2k = [
        nc.dram_tensor(f"w2_{k}", [128, D_OUT], F32R, kind="ExternalInput")
        for k in range(4)
    ]
    g1d = nc.dram_tensor("g1", [128, 4], F32, kind="ExternalInput")
    be1d = nc.dram_tensor("be1", [128, 4], F32, kind="ExternalInput")
    g2d = nc.dram_tensor("g2", [128, 8], F32, kind="ExternalInput")
    be2d = nc.dram_tensor("be2", [128, 8], F32, kind="ExternalInput")
    out_d = nc.dram_tensor("out", [rows, D_OUT], F32, kind="ExternalOutput")

    inv_n = 1.0 / seg_len

    with tile.TileContext(nc) as tc:
        with (
            tc.tile_pool(name="consts", bufs=1) as consts,
            tc.tile_pool(name="xtp", bufs=3) as xtp,
            tc.tile_pool(name="h1p", bufs=2) as h1p,
            tc.tile_pool(name="h2p", bufs=2) as h2p,
            tc.tile_pool(name="outp", bufs=3) as outp,
            tc.tile_pool(name="statp", bufs=2) as statp,
            tc.tile_pool(name="y1pool", bufs=2, space="PSUM") as y1pool,
            tc.tile_pool(name="y2pool", bufs=4, space="PSUM") as y2pool,
            tc.tile_pool(name="ptpool", bufs=2, space="PSUM") as ptpool,
        ):
            ident = consts.tile([128, 128], F32)
            make_identity(nc, ident)
            w1_sb = consts.tile([D_IN, D_MID], F32R)
            nc.sync.dma_start(out=w1_sb, in_=w1[:, :])
            w2_sb = []
            for k in range(4):
                t = consts.tile([128, D_OUT], F32R, tag=f"w2sb{k}", name=f"w2sb{k}")
                nc.sync.dma_start(out=t, in_=w2k[k][:, :])
                w2_sb.append(t)
            g1_sb = consts.tile([128, 4], F32)
            nc.sync.dma_start(out=g1_sb, in_=g1d[:, :])
            be1_sb = consts.tile([128, 4], F32)
            nc.sync.dma_start(out=be1_sb, in_=be1d[:, :])
            g2_sb = consts.tile([128, 8], F32)
            nc.sync.dma_start(out=g2_sb, in_=g2d[:, :])
            be2_sb = consts.tile([128, 8], F32)
            nc.sync.dma_start(out=be2_sb, in_=be2d[:, :])
            eps_sb = consts.tile([128, 1], F32)
            nc.vector.memset(eps_sb, EPS)

            def seg_bn_params(mv, M, g_sb, be_sb, pfx):
                """mv: [128, M*S*2] tile of (mean, var) pairs ->
                A = g*rsqrt(var+eps), B = be - mean*A, both [128, M*S]."""
                G = M * S
                mean_v = mv[:, :].rearrange("p (g two) -> p g two", two=2)[:, :, 0:1]
                var_v = mv[:, :].rearrange("p (g two) -> p g two", two=2)[:, :, 1:2]
                std = statp.tile([128, G, 1], F32, tag=f"{pfx}_std", name=f"{pfx}_std")
                nc.scalar.activation(
                    std[:, :, :], var_v, mybir.ActivationFunctionType.Sqrt,
                    bias=eps_sb[:, :],
                )
                rstd = statp.tile([128, G, 1], F32, tag=f"{pfx}_rstd", name=f"{pfx}_rstd")
                nc.vector.reciprocal(rstd[:, :, :], std[:, :, :])
                # g / be broadcast over segments: [128, M] -> [128, M, S, 1]
                gap = g_sb
                gb = bass.AP(
                    tensor=gap.tensor, offset=gap.offset,
                    ap=[gap.ap[0], gap.ap[1], [0, S], [0, 1]],
                )
                beap = be_sb
                beb = bass.AP(
                    tensor=beap.tensor, offset=beap.offset,
                    ap=[beap.ap[0], beap.ap[1], [0, S], [0, 1]],
                )
                a_t = statp.tile([128, G, 1], F32, tag=f"{pfx}_a", name=f"{pfx}_a")
                rstd_v = rstd[:, :, :].rearrange("p (m s) o -> p m s o", s=S)
                nc.vector.tensor_tensor(
                    out=a_t[:, :, :].rearrange("p (m s) o -> p m s o", s=S),
                    in0=rstd_v, in1=gb, op=mybir.AluOpType.mult,
                )
                ma = statp.tile([128, G, 1], F32, tag=f"{pfx}_ma", name=f"{pfx}_ma")
                nc.vector.tensor_tensor(
                    out=ma[:, :, :], in0=mean_v, in1=a_t[:, :, :],
                    op=mybir.AluOpType.mult,
                )
                b_t = statp.tile([128, G, 1], F32, tag=f"{pfx}_b", name=f"{pfx}_b")
                nc.vector.tensor_tensor(
                    out=b_t[:, :, :].rearrange("p (m s) o -> p m s o", s=S),
                    in0=beb,
                    in1=ma[:, :, :].rearrange("p (m s) o -> p m s o", s=S),
                    op=mybir.AluOpType.subtract,
                )
                return a_t, b_t

            for c in range(n_chunks):
                r0 = c * chunk
                xt = xtp.tile([D_IN, chunk], F32R, tag="xt", name=f"xt{c}")
                nc.sync.dma_start(out=xt[0:H_DIM, :], in_=hT[:, r0 : r0 + chunk])
                nc.sync.dma_start(out=xt[H_DIM:D_IN, :], in_=posT[:, r0 : r0 + chunk])

                # ---- layer 1 in groups of 2 m-tiles (y1pool bufs=2) ----
                h1t = []
                for grp in range(2):
                    ms = [2 * grp, 2 * grp + 1]
                    y1t = {}
                    for m in ms:
                        y1 = y1pool.tile([128, chunk], F32, tag="y1", name=f"y1_{c}_{m}")
                        nc.tensor.matmul(
                            y1[:, :], w1_sb[:, m * 128 : (m + 1) * 128], xt[:, :],
                            start=True, stop=True,
                        )
                        y1t[m] = y1
                    st1 = statp.tile([128, 2, S, 6], F32, tag="st1", name=f"st1_{c}_{grp}")
                    for j, m in enumerate(ms):
                        for s_i in range(S):
                            nc.vector.bn_stats(
                                st1[:, j : j + 1, s_i : s_i + 1, :],
                                y1t[m][:, s_i * seg_len : (s_i + 1) * seg_len],
                            )
                    mv1 = statp.tile([128, 2 * S * 2], F32, tag="mv1", name=f"mv1_{c}_{grp}")
                    for j in range(2):
                        for s in range(S):
                            i = j * S + s
                            nc.vector.bn_aggr(
                                mv1[:, 2 * i : 2 * i + 2], st1[:, j : j + 1, s : s + 1, :]
                            )
                    a1, b1 = seg_bn_params(
                        mv1, 2, g1_sb[:, 2 * grp : 2 * grp + 2],
                        be1_sb[:, 2 * grp : 2 * grp + 2], f"p1_{c}_{grp}"
                    )
                    for j, m in enumerate(ms):
                        h1 = h1p.tile([128, chunk], F32R, tag=f"h1_{m}", name=f"h1_{c}_{m}")
                        for s in range(S):
                            i = j * S + s
                            nc.scalar.activation(
                                h1[:, s * seg_len : (s + 1) * seg_len],
                                y1t[m][:, s * seg_len : (s + 1) * seg_len],
                                mybir.ActivationFunctionType.Relu,
                                bias=b1[:, i : i + 1, :], scale=a1[:, i : i + 1, :],
                            )
                        h1t.append(h1)

                # ---- layer 2 in groups of 4 m2-tiles (y2pool bufs=4) ----
                h2t = []
                for grp in range(2):
                    m2s = list(range(4 * grp, 4 * grp + 4))
                    y2t = {}
                    for m2 in m2s:
                        y2 = y2pool.tile([128, chunk], F32, tag="y2", name=f"y2_{c}_{m2}")
                        for k in range(4):
                            nc.tensor.matmul(
                                y2[:, :],
                                w2_sb[k][:, m2 * 128 : (m2 + 1) * 128],
                                h1t[k][:, :],
                                start=(k == 0), stop=(k == 3),
                            )
                        y2t[m2] = y2
                    st2 = statp.tile([128, 4, S, 6], F32, tag="st2", name=f"st2_{c}_{grp}")
                    for j, m2 in enumerate(m2s):
                        for s_i in range(S):
                            nc.vector.bn_stats(
                                st2[:, j : j + 1, s_i : s_i + 1, :],
                                y2t[m2][:, s_i * seg_len : (s_i + 1) * seg_len],
                            )
                    mv2 = statp.tile([128, 4 * S * 2], F32, tag="mv2", name=f"mv2_{c}_{grp}")
                    for j in range(4):
                        for s in range(S):
                            i = j * S + s
                            nc.vector.bn_aggr(
                                mv2[:, 2 * i : 2 * i + 2], st2[:, j : j + 1, s : s + 1, :]
                            )
                    a2, b2 = seg_bn_params(
                        mv2, 4, g2_sb[:, 4 * grp : 4 * grp + 4],
                        be2_sb[:, 4 * grp : 4 * grp + 4], f"p2_{c}_{grp}"
                    )
                    for j, m2 in enumerate(m2s):
                        h2 = h2p.tile([128, chunk], F32, tag=f"h2_{m2}", name=f"h2_{c}_{m2}")
                        for s in range(S):
                            i = j * S + s
                            nc.scalar.activation(
                                h2[:, s * seg_len : (s + 1) * seg_len],
                                y2t[m2][:, s * seg_len : (s + 1) * seg_len],
                                mybir.ActivationFunctionType.Relu,
                                bias=b2[:, i : i + 1, :], scale=a2[:, i : i + 1, :],
                            )
                        h2t.append(h2)

                # ---- transpose [1024, chunk] -> [chunk, 1024] and store ----
                for t in range(chunk // 128):
                    osb = outp.tile([128, D_OUT], F32, tag="osb", name=f"osb_{c}_{t}")
                    for half in range(2):
                        pt = ptpool.tile([128, 512], F32, tag="pt", name=f"pt_{c}_{t}_{half}")
                        for j in range(4):
                            m2 = half * 4 + j
                            nc.tensor.transpose(
                                pt[:, j * 128 : (j + 1) * 128],
                                h2t[m2][:, t * 128 : (t + 1) * 128],
                                ident[:, :],
                            )
                        if half == 0:
                            nc.scalar.copy(osb[:, 0:512], pt[:, :])
                        else:
                            nc.vector.tensor_copy(osb[:, 512:1024], pt[:, :])
                    nc.sync.dma_start(
                        out=out_d[r0 + t * 128 : r0 + (t + 1) * 128, :], in_=osb[:, :]
                    )

    nc.compile()
    return nc


def _prep_shared(W_sp, W1, W2, g1, be1, g2, be2):
    W1p = np.concatenate([W1[:, 32:], W1[:, :32] @ W_sp], axis=1)  # (512, 68)
    w1 = np.ascontiguousarray(W1p.T.astype(np.float32))            # (68, 512)
    W2T = np.ascontiguousarray(W2.T.astype(np.float32))            # (512, 1024)
    w2 = {f"w2_{k}": np.ascontiguousarray(W2T[k * 128 : (k + 1) * 128])
          for k in range(4)}
    packv = lambda v, m: np.ascontiguousarray(
        np.asarray(v, np.float32).reshape(m, 128).T
    )
    return {
        "w1": w1, **w2,
        "g1": packv(g1, 4), "be1": packv(be1, 4),
        "g2": packv(g2, 8), "be2": packv(be2, 8),
    }


def kernel(h_state, end_block_pos_rel, W_sp, b_sp, W1, b1, g1, be1, W2, b2, g2, be2,
           seq_start_end):
    h_state = np.asarray(h_state)
    ebp = np.asarray(end_block_pos_rel)
    sse = np.asarray(seq_start_end)
    total = h_state.shape[0]

    starts, ends = sse[:, 0], sse[:, 1]
    seg_lens = (ends - starts).astype(np.int64)
    contiguous = bool(starts[0] == 0 and ends[-1] == total and
                      np.all(starts[1:] == ends[:-1]))
    uniform = bool(contiguous and np.all(seg_lens == seg_lens[0]))
    nseg = len(seg_lens)

    if not uniform or total % (N_CORES * max(1, seg_lens[0])) or nseg % N_CORES:
        return _kernel_numpy(h_state, ebp, W_sp, b_sp, W1, b1, g1, be1,
                             W2, b2, g2, be2, sse)

    seg_len = int(seg_lens[0])
    rows = total // N_CORES

    # host-side input staging
    hT = np.ascontiguousarray(h_state.T.astype(np.float32))          # (64, total)
    # pos[n, bf*2+t] = ebp[n, t, bf]  ->  posT[(bf,t), n]
    posT = np.ascontiguousarray(
        np.transpose(ebp.astype(np.float32), (2, 1, 0)).reshape(4, total)
    )
    shared = _prep_shared(np.asarray(W_sp), np.asarray(W1), np.asarray(W2),
                          g1, be1, g2, be2)

    key = (rows, seg_len)
    if key not in _PROGRAM_CACHE:
        _PROGRAM_CACHE[key] = _build_program(rows, seg_len)
    nc = _PROGRAM_CACHE[key]

    in_maps = []
    for c in range(N_CORES):
        r0 = c * rows
        in_maps.append({
            "hT": np.ascontiguousarray(hT[:, r0 : r0 + rows]),
            "posT": np.ascontiguousarray(posT[:, r0 : r0 + rows]),
            **shared,
        })

    trace = os.environ.get("KERNEL_TRACE", "0") == "1"
    if trace:
        _install_ntff_hook_shim()
    res = bass_utils.run_bass_kernel_spmd(
        nc, in_maps, core_ids=list(range(N_CORES)), trace=trace,
    )
    LAST_RUN["exec_time_ns"] = res.exec_time_ns
    LAST_RUN["mean_exec_time_ns"] = res.mean_exec_time_ns
    LAST_RUN["trace"] = res.instructions_and_trace[1] if res.instructions_and_trace else None

    out = np.empty((total, D_OUT), np.float32)
    for c in range(N_CORES):
        out[c * rows : (c + 1) * rows] = res.results[c]["out"]
    return out


def _kernel_numpy(h_state, ebp, W_sp, b_sp, W1, b1, g1, be1, W2, b2, g2, be2, sse):
    """Correctness fallback for unsupported (ragged) segment layouts."""
    total = h_state.shape[0]
    pos = np.transpose(ebp, (0, 2, 1)).reshape(total, -1)
    rel = pos @ np.asarray(W_sp).T + np.asarray(b_sp)
    x = np.concatenate([rel, h_state], axis=1) @ np.asarray(W1).T + np.asarray(b1)

    def seg_bn_relu(v, g, be):
        o = np.empty_like(v)
        for s0, s1 in np.asarray(sse):
            blk = v[s0:s1]
            mu = blk.mean(axis=0)
            var = blk.var(axis=0)
            o[s0:s1] = np.maximum((blk - mu) / np.sqrt(var + EPS) * g + be, 0.0)
        return o

    h1 = seg_bn_relu(x, np.asarray(g1), np.asarray(be1))
    y2 = h1 @ np.asarray(W2).T + np.asarray(b2)
    return seg_bn_relu(y2, np.asarray(g2), np.asarray(be2)).astype(np.float32)


def _install_ntff_hook_shim():
    import types

    if "antenv.axon_hooks" in sys.modules:
        return
    try:
        import antenv

        mod = types.ModuleType("antenv.axon_hooks")
        _hook = [None]
        mod.set_axon_ntff_profile_hook = lambda h: _hook.__setitem__(0, h)
        mod.get_axon_ntff_profile_hook = lambda: _hook[0]
        sys.modules["antenv.axon_hooks"] = mod
        antenv.axon_hooks = mod
        from trn_agent_boot.trn_boot import _ntff_profile_via_ctypes

        mod.set_axon_ntff_profile_hook(
            _ntff_profile_via_ctypes("/opt/axon/libaxon_pjrt.so")
        )
    except Exception as e:
        print(f"ntff hook shim unavailable: {e}", file=sys.stderr)


# revision 9
# speedup vs baseline: 1.0349x; 1.0349x over previous
"""PoolBlockNet Trainium2 kernel.

reference computation (per row n, segment s(n)):
  pos    = permute(end_block_pos_rel)          # (N, 4)
  x      = [h_state | pos] @ W1p.T             # (N, 512)   W1p = [W1[:,32:] | W1[:,:32]@W_sp]
  h1     = relu(segBN(x; g1, be1))             # per-segment batchnorm (biased var)
  y2     = h1 @ W2.T                           # (N, 1024)
  out    = relu(segBN(y2; g2, be2))
Linear biases (b_sp, b1, b2) cancel inside segment BN and are dropped.

Strategy: data-parallel over whole segments across 8 cores.  On-chip layout is
feature-major ([feat, rows]) so BN stats are free-dim reductions; matmuls use
float32r (~1e-4 rounding, 1 cycle/row); the final [1024, rows] -> [rows, 1024]
transpose is done on the PE (exact fp32) and written out contiguously.
"""

import math
import os
import sys

import numpy as np

sys.path.insert(0, "/opt/trn_rl_repo")

import concourse.bacc as bacc
import concourse.bass as bass
import concourse.mybir as mybir
import concourse.tile as tile
from concourse import bass_utils
from concourse.masks import make_identity

F32 = mybir.dt.float32
F32R = mybir.dt.float32r
EPS = 1e-5
H_DIM = 64
D_IN = 68          # 64 h_state + 4 pos features, contraction dim of layer 1
D_MID = 512
D_OUT = 1024
N_CORES = 8
GROUPED_BN_STATS = os.environ.get("KERNEL_GROUPED_BN", "0") == "1"

# filled by kernel() for the benchmark harness (exec_time_ns etc.)
LAST_RUN = {}

_PROGRAM_CACHE = {}


def _build_program(rows, seg_len):
    """One-core SPMD program: rows rows, uniform segments of seg_len."""
    if seg_len <= 256:
        assert 512 % seg_len == 0
        chunk = 512
    else:
        assert seg_len <= 512
        chunk = seg_len
    S = chunk // seg_len               # segments per chunk
    assert rows % chunk == 0
    n_chunks = rows // chunk
    G1 = 4 * S                         # (m-tile, seg) groups for layer 1
    G2 = 8 * S

    nc = bacc.Bacc("TRN2", target_bir_lowering=False, debug=False)

    hT = nc.dram_tensor("hT", [H_DIM, rows], F32R, kind="ExternalInput")
    posT = nc.dram_tensor("posT", [4, rows], F32R, kind="ExternalInput")
    w1 = nc.dram_tensor("w1", [D_IN, D_MID], F32R, kind="ExternalInput")
    w2k = [
        nc.dram_tensor(f"w2_{k}", [128, D_OUT], F32R, kind="ExternalInput")
        for k in range(4)
    ]
    g1d = nc.dram_tensor("g1", [128, 4], F32, kind="ExternalInput")
    be1d = nc.dram_tensor("be1", [128, 4], F32, kind="ExternalInput")
    g2d = nc.dram_tensor("g2", [128, 8], F32, kind="ExternalInput")
    be2d = nc.dram_tensor("be2", [128, 8], F32, kind="ExternalInput")
    out_d = nc.dram_tensor("out", [rows, D_OUT], F32, kind="ExternalOutput")

    inv_n = 1.0 / seg_len

    with tile.TileContext(nc) as tc:
        with (
            tc.tile_pool(name="consts", bufs=1) as consts,
            tc.tile_pool(name="xtp", bufs=4) as xtp,
            tc.tile_pool(name="h1p", bufs=3) as h1p,
            tc.tile_pool(name="h2p", bufs=3) as h2p,
            tc.tile_pool(name="outp", bufs=4) as outp,
            tc.tile_pool(name="statp", bufs=3) as statp,
            tc.tile_pool(name="y1pool", bufs=2, space="PSUM") as y1pool,
            tc.tile_pool(name="y2pool", bufs=4, space="PSUM") as y2pool,
            tc.tile_pool(name="ptpool", bufs=2, space="PSUM") as ptpool,
        ):
            ident = consts.tile([128, 128], F32)
            make_identity(nc, ident)
            ident_r = consts.tile([128, 128], F32R)
            nc.vector.tensor_copy(ident_r[:, :], ident[:, :])
            w1_sb = consts.tile([D_IN, D_MID], F32R)
            nc.sync.dma_start(out=w1_sb, in_=w1[:, :])
            w2_sb = []
            for k in range(4):
                t = consts.tile([128, D_OUT], F32R, tag=f"w2sb{k}", name=f"w2sb{k}")
                nc.sync.dma_start(out=t, in_=w2k[k][:, :])
                w2_sb.append(t)
            g1_sb = consts.tile([128, 4], F32)
            nc.sync.dma_start(out=g1_sb, in_=g1d[:, :])
            be1_sb = consts.tile([128, 4], F32)
            nc.sync.dma_start(out=be1_sb, in_=be1d[:, :])
            g2_sb = consts.tile([128, 8], F32)
            nc.sync.dma_start(out=g2_sb, in_=g2d[:, :])
            be2_sb = consts.tile([128, 8], F32)
            nc.sync.dma_start(out=be2_sb, in_=be2d[:, :])
            eps_sb = consts.tile([128, 1], F32)
            nc.vector.memset(eps_sb, EPS)

            def bn_stats_grouped(out_ap, in_ap):
                # multi-group bn_stats: stop the AP optimizer from merging the
                # (group, elem) dims so the DVE emits one 6-tuple per group
                return nc.vector.add_instruction(
                    mybir.InstBNStats(
                        name=nc.get_next_instruction_name(),
                        ins=[nc.vector.lower_ap(in_ap, opt=False)],
                        outs=[nc.vector.lower_ap(out_ap, opt=False)],
                    )
                )

            def seg_bn_params(mv, M, g_sb, be_sb, tag, pfx):
                """mv: [128, M*S*2] tile of (mean, var) pairs ->
                A = g*rsqrt(var+eps), B = be - mean*A, both [128, M*S]."""
                G = M * S
                mean_v = mv[:, :].rearrange("p (g two) -> p g two", two=2)[:, :, 0:1]
                var_v = mv[:, :].rearrange("p (g two) -> p g two", two=2)[:, :, 1:2]
                std = statp.tile([128, G, 1], F32, tag=f"{tag}_std", name=f"{pfx}_std")
                nc.scalar.activation(
                    std[:, :, :], var_v, mybir.ActivationFunctionType.Sqrt,
                    bias=eps_sb[:, :],
                )
                rstd = statp.tile([128, G, 1], F32, tag=f"{tag}_rstd", name=f"{pfx}_rstd")
                nc.vector.reciprocal(rstd[:, :, :], std[:, :, :])
                # g / be broadcast over segments: [128, M] -> [128, M, S, 1]
                gap = g_sb
                gb = bass.AP(
                    tensor=gap.tensor, offset=gap.offset,
                    ap=[gap.ap[0], gap.ap[1], [0, S], [0, 1]],
                )
                beap = be_sb
                beb = bass.AP(
                    tensor=beap.tensor, offset=beap.offset,
                    ap=[beap.ap[0], beap.ap[1], [0, S], [0, 1]],
                )
                a_t = statp.tile([128, G, 1], F32, tag=f"{tag}_a", name=f"{pfx}_a")
                rstd_v = rstd[:, :, :].rearrange("p (m s) o -> p m s o", s=S)
                nc.vector.tensor_tensor(
                    out=a_t[:, :, :].rearrange("p (m s) o -> p m s o", s=S),
                    in0=rstd_v, in1=gb, op=mybir.AluOpType.mult,
                )
                ma = statp.tile([128, G, 1], F32, tag=f"{tag}_ma", name=f"{pfx}_ma")
                nc.vector.tensor_tensor(
                    out=ma[:, :, :], in0=mean_v, in1=a_t[:, :, :],
                    op=mybir.AluOpType.mult,
                )
                b_t = statp.tile([128, G, 1], F32, tag=f"{tag}_b", name=f"{pfx}_b")
                nc.vector.tensor_tensor(
                    out=b_t[:, :, :].rearrange("p (m s) o -> p m s o", s=S),
                    in0=beb,
                    in1=ma[:, :, :].rearrange("p (m s) o -> p m s o", s=S),
                    op=mybir.AluOpType.subtract,
                )
                return a_t, b_t

            for c in range(n_chunks):
                r0 = c * chunk
                xt = xtp.tile([D_IN, chunk], F32R, tag="xt", name=f"xt{c}")
                nc.sync.dma_start(out=xt[0:H_DIM, :], in_=hT[:, r0 : r0 + chunk])
                nc.sync.dma_start(out=xt[H_DIM:D_IN, :], in_=posT[:, r0 : r0 + chunk])

                # ---- layer 1 in groups of 2 m-tiles (y1pool bufs=2) ----
                h1t = []
                for grp in range(2):
                    ms = [2 * grp, 2 * grp + 1]
                    y1t = {}
                    for m in ms:
                        y1 = y1pool.tile([128, chunk], F32, tag="y1", name=f"y1_{c}_{m}")
                        nc.tensor.matmul(
                            y1[:, :], w1_sb[:, m * 128 : (m + 1) * 128], xt[:, :],
                            start=True, stop=True,
                        )
                        y1t[m] = y1
                    st1 = statp.tile([128, 2, S, 6], F32, tag="st1", name=f"st1_{c}_{grp}")
                    for j, m in enumerate(ms):
                        if GROUPED_BN_STATS and S > 1:
                            bn_stats_grouped(
                                st1[:, j : j + 1, :, :].rearrange(
                                    "p a s x -> p (a s) x"
                                ),
                                y1t[m][:, :].rearrange("p (s n) -> p s n", s=S),
                            )
                        else:
                            for s_i in range(S):
                                nc.vector.bn_stats(
                                    st1[:, j : j + 1, s_i : s_i + 1, :],
                                    y1t[m][:, s_i * seg_len : (s_i + 1) * seg_len],
                                )
                    mv1 = statp.tile([128, 2 * S * 2], F32, tag="mv1", name=f"mv1_{c}_{grp}")
                    for j in range(2):
                        for s in range(S):
                            i = j * S + s
                            nc.vector.bn_aggr(
                                mv1[:, 2 * i : 2 * i + 2], st1[:, j : j + 1, s : s + 1, :]
                            )
                    a1, b1 = seg_bn_params(
                        mv1, 2, g1_sb[:, 2 * grp : 2 * grp + 2],
                        be1_sb[:, 2 * grp : 2 * grp + 2], "p1", f"p1_{c}_{grp}"
                    )
                    for j, m in enumerate(ms):
                        h1 = h1p.tile([128, chunk], F32R, tag=f"h1_{m}", name=f"h1_{c}_{m}")
                        for s in range(S):
                            i = j * S + s
                            nc.scalar.activation(
                                h1[:, s * seg_len : (s + 1) * seg_len],
                                y1t[m][:, s * seg_len : (s + 1) * seg_len],
                                mybir.ActivationFunctionType.Relu,
                                bias=b1[:, i : i + 1, :], scale=a1[:, i : i + 1, :],
                            )
                        h1t.append(h1)

                # ---- layer 2 in groups of 4 m2-tiles (y2pool bufs=4) ----
                h2t = []
                for grp in range(2):
                    m2s = list(range(4 * grp, 4 * grp + 4))
                    y2t = {}
                    for m2 in m2s:
                        y2 = y2pool.tile([128, chunk], F32, tag="y2", name=f"y2_{c}_{m2}")
                        for k in range(4):
                            nc.tensor.matmul(
                                y2[:, :],
                                w2_sb[k][:, m2 * 128 : (m2 + 1) * 128],
                                h1t[k][:, :],
                                start=(k == 0), stop=(k == 3),
                            )
                        y2t[m2] = y2
                    st2 = statp.tile([128, 4, S, 6], F32, tag="st2", name=f"st2_{c}_{grp}")
                    for j, m2 in enumerate(m2s):
                        if GROUPED_BN_STATS and S > 1:
                            bn_stats_grouped(
                                st2[:, j : j + 1, :, :].rearrange(
                                    "p a s x -> p (a s) x"
                                ),
                                y2t[m2][:, :].rearrange("p (s n) -> p s n", s=S),
                            )
                        else:
                            for s_i in range(S):
                                nc.vector.bn_stats(
                                    st2[:, j : j + 1, s_i : s_i + 1, :],
                                    y2t[m2][:, s_i * seg_len : (s_i + 1) * seg_len],
                                )
                    mv2 = statp.tile([128, 4 * S * 2], F32, tag="mv2", name=f"mv2_{c}_{grp}")
                    for j in range(4):
                        for s in range(S):
                            i = j * S + s
                            nc.vector.bn_aggr(
                                mv2[:, 2 * i : 2 * i + 2], st2[:, j : j + 1, s : s + 1, :]
                            )
                    a2, b2 = seg_bn_params(
                        mv2, 4, g2_sb[:, 4 * grp : 4 * grp + 4],
                        be2_sb[:, 4 * grp : 4 * grp + 4], "p2", f"p2_{c}_{grp}"
                    )
                    for j, m2 in enumerate(m2s):
                        h2 = h2p.tile([128, chunk], F32R, tag=f"h2_{m2}", name=f"h2_{c}_{m2}")
                        for s in range(S):
                            i = j * S + s
                            nc.scalar.activation(
                                h2[:, s * seg_len : (s + 1) * seg_len],
                                y2t[m2][:, s * seg_len : (s + 1) * seg_len],
                                mybir.ActivationFunctionType.Relu,
                                bias=b2[:, i : i + 1, :], scale=a2[:, i : i + 1, :],
                            )
                        h2t.append(h2)

                # ---- transpose [1024, chunk] -> [chunk, 1024] and store ----
                for t in range(chunk // 128):
                    osb = outp.tile([128, D_OUT], F32, tag="osb", name=f"osb_{c}_{t}")
                    for half in range(2):
                        pt = ptpool.tile([128, 512], F32, tag="pt", name=f"pt_{c}_{t}_{half}")
                        for j in range(4):
                            m2 = half * 4 + j
                            nc.tensor.transpose(
                                pt[:, j * 128 : (j + 1) * 128].bitcast(F32R),
                                h2t[m2][:, t * 128 : (t + 1) * 128],
                                ident_r[:, :],
                            )
                        if half == 0:
                            nc.scalar.copy(osb[:, 0:512], pt[:, :])
                        else:
                            nc.vector.tensor_copy(osb[:, 512:1024], pt[:, :])
                    nc.sync.dma_start(
                        out=out_d[r0 + t * 128 : r0 + (t + 1) * 128, :], in_=osb[:, :]
                    )

    nc.compile()
    return nc


def _prep_shared(W_sp, W1, W2, g1, be1, g2, be2):
    W1p = np.concatenate([W1[:, 32:], W1[:, :32] @ W_sp], axis=1)  # (512, 68)
    w1 = np.ascontiguousarray(W1p.T.astype(np.float32))            # (68, 512)
    W2T = np.ascontiguousarray(W2.T.astype(np.float32))            # (512, 1024)
    w2 = {f"w2_{k}": np.ascontiguousarray(W2T[k * 128 : (k + 1) * 128])
          for k in range(4)}
    packv = lambda v, m: np.ascontiguousarray(
        np.asarray(v, np.float32).reshape(m, 128).T
    )
    return {
        "w1": w1, **w2,
        "g1": packv(g1, 4), "be1": packv(be1, 4),
        "g2": packv(g2, 8), "be2": packv(be2, 8),
    }


def kernel(h_state, end_block_pos_rel, W_sp, b_sp, W1, b1, g1, be1, W2, b2, g2, be2,
           seq_start_end):
    h_state = np.asarray(h_state)
    ebp = np.asarray(end_block_pos_rel)
    sse = np.asarray(seq_start_end)
    total = h_state.shape[0]

    starts, ends = sse[:, 0], sse[:, 1]
    seg_lens = (ends - starts).astype(np.int64)
    contiguous = bool(starts[0] == 0 and ends[-1] == total and
                      np.all(starts[1:] == ends[:-1]))
    uniform = bool(contiguous and np.all(seg_lens == seg_lens[0]))
    nseg = len(seg_lens)

    if not uniform or total % (N_CORES * max(1, seg_lens[0])) or nseg % N_CORES:
        return _kernel_numpy(h_state, ebp, W_sp, b_sp, W1, b1, g1, be1,
                             W2, b2, g2, be2, sse)

    seg_len = int(seg_lens[0])
    rows = total // N_CORES

    # host-side input staging
    hT = np.ascontiguousarray(h_state.T.astype(np.float32))          # (64, total)
    # pos[n, bf*2+t] = ebp[n, t, bf]  ->  posT[(bf,t), n]
    posT = np.ascontiguousarray(
        np.transpose(ebp.astype(np.float32), (2, 1, 0)).reshape(4, total)
    )
    shared = _prep_shared(np.asarray(W_sp), np.asarray(W1), np.asarray(W2),
                          g1, be1, g2, be2)

    key = (rows, seg_len)
    if key not in _PROGRAM_CACHE:
        _PROGRAM_CACHE[key] = _build_program(rows, seg_len)
    nc = _PROGRAM_CACHE[key]

    in_maps = []
    for c in range(N_CORES):
        r0 = c * rows
        in_maps.append({
            "hT": np.ascontiguousarray(hT[:, r0 : r0 + rows]),
            "posT": np.ascontiguousarray(posT[:, r0 : r0 + rows]),
            **shared,
        })

    trace = os.environ.get("KERNEL_TRACE", "0") == "1"
    if trace:
        _install_ntff_hook_shim()
    res = bass_utils.run_bass_kernel_spmd(
        nc, in_maps, core_ids=list(range(N_CORES)), trace=trace,
    )
    LAST_RUN["exec_time_ns"] = res.exec_time_ns
    LAST_RUN["mean_exec_time_ns"] = res.mean_exec_time_ns
    LAST_RUN["trace"] = res.instructions_and_trace[1] if res.instructions_and_trace else None

    out = np.empty((total, D_OUT), np.float32)
    for c in range(N_CORES):
        out[c * rows : (c + 1) * rows] = res.results[c]["out"]
    return out


def _kernel_numpy(h_state, ebp, W_sp, b_sp, W1, b1, g1, be1, W2, b2, g2, be2, sse):
    """Correctness fallback for unsupported (ragged) segment layouts."""
    total = h_state.shape[0]
    pos = np.transpose(ebp, (0, 2, 1)).reshape(total, -1)
    rel = pos @ np.asarray(W_sp).T + np.asarray(b_sp)
    x = np.concatenate([rel, h_state], axis=1) @ np.asarray(W1).T + np.asarray(b1)

    def seg_bn_relu(v, g, be):
        o = np.empty_like(v)
        for s0, s1 in np.asarray(sse):
            blk = v[s0:s1]
            mu = blk.mean(axis=0)
            var = blk.var(axis=0)
            o[s0:s1] = np.maximum((blk - mu) / np.sqrt(var + EPS) * g + be, 0.0)
        return o

    h1 = seg_bn_relu(x, np.asarray(g1), np.asarray(be1))
    y2 = h1 @ np.asarray(W2).T + np.asarray(b2)
    return seg_bn_relu(y2, np.asarray(g2), np.asarray(be2)).astype(np.float32)


def _install_ntff_hook_shim():
    import types

    if "antenv.axon_hooks" in sys.modules:
        return
    try:
        import antenv

        mod = types.ModuleType("antenv.axon_hooks")
        _hook = [None]
        mod.set_axon_ntff_profile_hook = lambda h: _hook.__setitem__(0, h)
        mod.get_axon_ntff_profile_hook = lambda: _hook[0]
        sys.modules["antenv.axon_hooks"] = mod
        antenv.axon_hooks = mod
        from trn_agent_boot.trn_boot import _ntff_profile_via_ctypes

        mod.set_axon_ntff_profile_hook(
            _ntff_profile_via_ctypes("/opt/axon/libaxon_pjrt.so")
        )
    except Exception as e:
        print(f"ntff hook shim unavailable: {e}", file=sys.stderr)
